# revision 1
# baseline (speedup 1.0000x reference)
"""Trainium2 Bass kernel for nn_AttentionCropLayer (attention crop + bilinear
resize), data-parallel over 8 NeuronCores.

Reformulation (validated vs the jax reference, rel ~3e-3 in bf16):
  For each sample, the soft-masked crop + align-corners bilinear resize is
  exactly  out[c] = Rt.T @ X[c] @ Ct  with
    Rt[i,j] = mrow[i] * hat(i - sr[j]),  Ct[k,m] = mcol[k] * hat(k - sc[m]),
    hat(d) = relu(1 - |d|),
    sr[j] = w_off + j*(w_end-w_off-1)/107  (and likewise sc),
  because the reference's integer crop box satisfies w_off >= 26 > 0 so the
  r0/r1 gather taps are exactly the two nonzeros of the hat function, and the
  sigmoid box masks fold into the interpolation matrices diagonally.

  trunc(m*l + 0.5) is computed as the hardware f32->int32 convert of m*l
  (round-to-nearest-even == trunc(x+0.5) except at measure-zero ties).

Per 16-sample slab: DMA f32 -> bf16 convert -> hat build (DVE) ->
PE transpose of hat -> ACT relu(scale=-mask) -> mm1 (X_c stationary) ->
T1 copy (bf16) -> mm2 -> ACT copy -> DMA out. All matmuls bf16, PSUM f32.
"""
import numpy as np
import ml_dtypes

import concourse.bass as bass
import concourse.tile as tile
from concourse import mybir
from concourse.alu_op_type import AluOpType as Op

F32 = mybir.dt.float32
BF16 = mybir.dt.bfloat16
I32 = mybir.dt.int32
AF = mybir.ActivationFunctionType
P = 108
N_CORES = 8
S = 128   # samples per core
SL = 16   # slab size

_ctr = [0]


def _split_multi_waits(nc):
    """This container's walrus accepts at most ONE sync-wait per instruction
    (none on Drain). Move excess waits onto preceding same-engine no-ops."""
    moved = 0
    for func in nc.m.functions:
        for blk in func.blocks:
            out_insts = []
            changed = False
            for inst in blk.instructions:
                si = inst.sync_info
                waits = list(si.on_wait) if (si and si.on_wait) else []
                limit = 0 if inst.opcode == "Drain" else 1
                if len(waits) > limit:
                    keep, excess = waits[:limit], waits[limit:]
                    for w in excess:
                        _ctr[0] += 1
                        nop = mybir.InstNoOp(
                            name=f"waitsplit-{_ctr[0]}",
                            sync_info=mybir.SyncInfo(on_wait=[w], on_update=[]),
                            bass_nofuse=True,
                            engine=inst.engine,
                        )
                        out_insts.append(nop)
                        moved += 1
                    upd = list(si.on_update) if si.on_update else []
                    inst.sync_info = mybir.SyncInfo(on_wait=keep, on_update=upd)
                    changed = True
                out_insts.append(inst)
            if changed:
                try:
                    blk.instructions = out_insts
                except Exception:
                    blk.clear_instructions()
                    for i in out_insts:
                        blk.add_instruction(i)
    return moved


def _build():
    nslabs = S // SL
    nc = bass.Bass()
    images = nc.declare_dram_parameter("images", [S, 3, P, P], F32, isOutput=False)
    locs = nc.declare_dram_parameter("locs", [S, 3], F32, isOutput=False)
    iota_d = nc.declare_dram_parameter("iota", [128, P], F32, isOutput=False)
    idf_d = nc.declare_dram_parameter("idf", [128, 128], F32, isOutput=False)
    idb_d = nc.declare_dram_parameter("idb", [128, 128], BF16, isOutput=False)
    out = nc.declare_dram_parameter("out", [S, 3, P, P], F32, isOutput=True)

    with tile.TileContext(nc) as tc:
        with (
            tc.tile_pool(name="consts", bufs=1) as consts,
            tc.tile_pool(name="setup", bufs=1) as setup,
            tc.tile_pool(name="setup_ps", bufs=2, space="PSUM") as setup_ps,
            tc.tile_pool(name="slab", bufs=2) as slab_pool,
            tc.tile_pool(name="samp", bufs=3) as samp,
            tc.tile_pool(name="ps_tr", bufs=2, space="PSUM") as ps_tr,
            tc.tile_pool(name="ps_mm", bufs=2, space="PSUM") as ps_mm,
        ):
            iota = consts.tile([128, P], F32)
            nc.sync.dma_start(out=iota, in_=iota_d[:, :])
            idf = consts.tile([128, 128], F32)
            nc.sync.dma_start(out=idf, in_=idf_d[:, :])
            idb = consts.tile([128, 128], BF16)
            nc.sync.dma_start(out=idb, in_=idb_d[:, :])

            lt = setup.tile([S, 3], F32)
            nc.sync.dma_start(out=lt, in_=locs[:, :])

            def col(t, j):
                return t[:, j:j + 1]

            # trunc(m*l + 0.5) == RNE-convert(m*l)
            tx = setup.tile([S, 1], F32)
            ty = setup.tile([S, 1], F32)
            tlh = setup.tile([S, 1], F32)
            for j, m, t in ((0, 27.0, tx), (1, 27.0, ty), (2, 7.0, tlh)):
                v = setup.tile([S, 1], F32, tag="v_scaled")
                nc.vector.tensor_scalar(v, col(lt, j), m, None, Op.mult)
                vi = setup.tile([S, 1], I32, tag="v_int")
                nc.vector.tensor_copy(vi, v)
                nc.vector.tensor_copy(t, vi)

            # w_off = tx - tlh + 33 ; w_end = min(tx + tlh + 75, 108)
            w_off = setup.tile([S, 1], F32)
            nc.vector.scalar_tensor_tensor(w_off, tx, 33.0, tlh, Op.add, Op.subtract)
            w_end = setup.tile([S, 1], F32)
            nc.vector.scalar_tensor_tensor(w_end, tx, 75.0, tlh, Op.add, Op.add)
            nc.vector.tensor_scalar(w_end, w_end, 108.0, None, Op.min)
            h_off = setup.tile([S, 1], F32)
            nc.vector.scalar_tensor_tensor(h_off, ty, 33.0, tlh, Op.add, Op.subtract)
            h_end = setup.tile([S, 1], F32)
            nc.vector.scalar_tensor_tensor(h_end, ty, 75.0, tlh, Op.add, Op.add)
            nc.vector.tensor_scalar(h_end, h_end, 108.0, None, Op.min)

            # sr = iota * (w_end-w_off-1)/107 + w_off
            sr = setup.tile([S, P], F32)
            sc = setup.tile([S, P], F32)
            for off, end, dst in ((w_off, w_end, sr), (h_off, h_end, sc)):
                a = setup.tile([S, 1], F32, tag="a_slope")
                nc.vector.scalar_tensor_tensor(a, end, -1.0, off, Op.add, Op.subtract)
                nc.vector.tensor_scalar(a, a, 1.0 / 107.0, None, Op.mult)
                nc.vector.tensor_scalar(dst, iota[:S, :], a, off, Op.mult, Op.add)

            # negated masks: m_neg = sig(10(i-end)) - sig(10(i-off))
            mrow_n = setup.tile([S, P], F32)
            mcol_n = setup.tile([S, P], F32)
            for off, end, dst in ((w_off, w_end, mrow_n), (h_off, h_end, mcol_n)):
                b_off = setup.tile([S, 1], F32, tag="b_off")
                nc.vector.tensor_scalar(b_off, off, -10.0, None, Op.mult)
                b_end = setup.tile([S, 1], F32, tag="b_end")
                nc.vector.tensor_scalar(b_end, end, -10.0, None, Op.mult)
                s_off = setup.tile([S, P], F32, tag="s_off")
                nc.scalar.activation(s_off, iota[:S, :], AF.Sigmoid, bias=b_off, scale=10.0)
                s_end = setup.tile([S, P], F32, tag="s_end")
                nc.scalar.activation(s_end, iota[:S, :], AF.Sigmoid, bias=b_end, scale=10.0)
                nc.vector.tensor_sub(dst, s_end, s_off)

            srT = setup.tile([P, S], F32)
            scT = setup.tile([P, S], F32)
            mrowT_n = setup.tile([P, S], F32)
            mcolT_n = setup.tile([P, S], F32)
            for src_t, dst in ((sr, srT), (sc, scT), (mrow_n, mrowT_n), (mcol_n, mcolT_n)):
                pst = setup_ps.tile([P, S], F32, tag="setup_tr")
                nc.tensor.transpose(pst, src_t, idf[:S, :S])
                nc.vector.tensor_copy(dst, pst)

            for t in range(nslabs):
                s0 = t * SL
                x_f32 = slab_pool.tile([P, SL, 3, P], F32, tag="x_f32")
                nc.sync.dma_start(out=x_f32,
                                  in_=images[s0:s0 + SL, :, :, :].transpose([2, 0, 1, 3]))
                x_b = slab_pool.tile([P, SL, 3, P], BF16, tag="x_b")

                # hat build: A[j,(s,i)] = |i - sr[j]| - 1 = max(d-1, -d-1), bf16
                a_r = slab_pool.tile([P, SL, P], BF16, tag="a_r")
                a_c = slab_pool.tile([P, SL, P], BF16, tag="a_c")
                d_r = slab_pool.tile([P, SL, P], F32, tag="d_r")
                d_c = slab_pool.tile([P, SL, P], F32, tag="d_c")
                e_r = slab_pool.tile([P, SL, P], F32, tag="e_r")
                e_c = slab_pool.tile([P, SL, P], F32, tag="e_c")
                iota_b = iota[:P, :].unsqueeze(1).broadcast_to([P, SL, P])
                srT_b = srT[:, s0:s0 + SL].unsqueeze(2).broadcast_to([P, SL, P])
                scT_b = scT[:, s0:s0 + SL].unsqueeze(2).broadcast_to([P, SL, P])
                nc.gpsimd.tensor_sub(d_r, iota_b, srT_b)
                nc.vector.tensor_sub(d_c, iota_b, scT_b)
                nc.gpsimd.tensor_scalar(e_r, d_r, -1.0, -1.0, Op.mult, Op.add)
                nc.vector.tensor_scalar(e_c, d_c, -1.0, -1.0, Op.mult, Op.add)
                nc.vector.scalar_tensor_tensor(a_r, d_r, -1.0, e_r, Op.add, Op.max)
                nc.vector.scalar_tensor_tensor(a_c, d_c, -1.0, e_c, Op.add, Op.max)

                o_stage = slab_pool.tile([P, SL, 3, P], F32, tag="o_stage")

                for sl in range(SL):
                    s = s0 + sl
                    nc.gpsimd.tensor_copy(x_b[:, sl], x_f32[:, sl])

                    rt_ps = ps_tr.tile([P, P], BF16, tag="tr")
                    nc.tensor.transpose(rt_ps, a_r[:, sl], idb[:P, :P])
                    rt = samp.tile([P, P], BF16, tag="rt")
                    nc.scalar.activation(rt, rt_ps, AF.Relu, scale=mrowT_n[:, s:s + 1])

                    ct_ps = ps_tr.tile([P, P], BF16, tag="tr")
                    nc.tensor.transpose(ct_ps, a_c[:, sl], idb[:P, :P])
                    ct = samp.tile([P, P], BF16, tag="ct")
                    nc.scalar.activation(ct, ct_ps, AF.Relu, scale=mcolT_n[:, s:s + 1])

                    t1_ps = ps_mm.tile([P, 3, P], F32, tag="t1")
                    for c in range(3):
                        nc.tensor.matmul(t1_ps[:, c], x_b[:, sl, c], rt,
                                         start=True, stop=True)
                    t1 = samp.tile([P, 3, P], BF16, tag="t1sb")
                    nc.vector.tensor_copy(t1, t1_ps)

                    o_ps = ps_mm.tile([P, 3, P], F32, tag="o")
                    for c in range(3):
                        nc.tensor.matmul(o_ps[:, c], t1[:, c], ct,
                                         start=True, stop=True)
                    nc.scalar.activation(o_stage[:, sl], o_ps, AF.Copy)

                nc.sync.dma_start(out=out[s0:s0 + SL, :, :, :].transpose([2, 0, 1, 3]),
                                  in_=o_stage)
    return nc


def _host_constants():
    iota = np.tile(np.arange(P, dtype=np.float32), (128, 1))
    idf = np.eye(128, dtype=np.float32)
    idb = np.eye(128, dtype=ml_dtypes.bfloat16)
    return {"iota": iota, "idf": idf, "idb": idb}


_cached_nc = None


def _get_nc():
    global _cached_nc
    if _cached_nc is None:
        nc = _build()
        _split_multi_waits(nc)
        _cached_nc = nc
    return _cached_nc


def kernel(images: np.ndarray, locs: np.ndarray) -> np.ndarray:
    from concourse.bass_utils import run_bass_kernel_spmd

    images = np.ascontiguousarray(np.asarray(images, dtype=np.float32))
    locs = np.ascontiguousarray(np.asarray(locs, dtype=np.float32))
    assert images.shape == (N_CORES * S, 3, P, P), images.shape
    assert locs.shape == (N_CORES * S, 3), locs.shape

    nc = _get_nc()
    consts = _host_constants()
    in_maps = [
        {
            "images": images[c * S:(c + 1) * S],
            "locs": locs[c * S:(c + 1) * S],
            **consts,
        }
        for c in range(N_CORES)
    ]
    res = run_bass_kernel_spmd(nc, in_maps, list(range(N_CORES)))
    return np.concatenate(
        [res.results[c]["out"] for c in range(N_CORES)], axis=0
    ).astype(np.float32)
